# revision 3
# baseline (speedup 1.0000x reference)
"""MoE layer (E=8, top-2, SwiGLU experts) on 8 trn2 NeuronCores.

Strategy (expert parallel, host-routed):
  - Router (flat @ router_w.T, top-2, softmax) is computed on host in fp32;
    it is tiny (33 MFLOP) and must match the reference's expert selection
    exactly (min top2-vs-3rd logit gap on these inputs is ~1e-4, far above
    fp32 matmul noise ~1e-6).
  - Tokens are dispatched to core e = expert e (the "all-to-all"), padded to
    a fixed capacity CAP. Each core runs a dense bf16 SwiGLU FFN for its
    expert over its routed tokens: yT = w2T.T @ (silu(w1T.T@xT) * (w3T.T@xT)).
    All tensors are pre-transposed AND pre-packed on host into the exact
    SBUF-resident layouts (partition-major, pack-contiguous) so every device
    DMA is a pure linear copy.
  - Host combines: out[tok] += combine_weight * y (each token appears in
    exactly 2 experts' outputs).

Compute dtype bf16 (PE runs fp32 at 1/4 rate), fp32 PSUM accumulation,
fp32 output.
"""

import os
import numpy as np
import ml_dtypes

B, S, D, H, E = 2, 2048, 1024, 2048, 8
T = B * S
TOP_K = 2
P = 128
NTOK = 512    # max token chunk (matmul free dim / one PSUM bank of fp32)
D_T = D // P  # 8 contraction slabs for stage 1 / output slabs for stage 2
H_T = H // P  # 16 hidden slabs
# w1/w3 stream in hidden-column packs: (h_start, h_count). The first packs
# are single slabs so the first gate group's critical DMA prefix is small.
PACKS13 = [(0, 1), (1, 1)] + [(2 + 2 * i, 2) for i in range(7)]
W2Q = 8       # w2 pack = 8 hidden slabs -> 2 packs

_cache = {}

# set by the last kernel() call when tracing is enabled (KERNEL_TRACE=1)
LAST_RESULTS = None


def _chunk_sizes(cap):
    """First chunk as large as possible (its stage 1 overlaps the weight
    stream, and a larger free dim slows per-h weight consumption below the
    DMA ring bandwidth); remainder split equally (multiples of 8)."""
    first = min(NTOK, cap)
    sizes = [first]
    rem = cap - first
    if rem:
        k = -(-rem // NTOK)
        base, r8 = divmod(rem // 8, k)
        sizes += [(base + (1 if i < r8 else 0)) * 8 for i in range(k)]
    chunks, s = [], 0
    for n in sizes:
        chunks.append((s, n))
        s += n
    return chunks


def _pack_x(xTe, chunks):
    """[D, cap] -> [128, D_T*cap], chunk-blocked, partition-major."""
    arr = xTe.reshape(D_T, P, -1).transpose(1, 0, 2)  # [128, D_T, cap]
    blocks = [arr[:, :, s0:s0 + n].reshape(P, D_T * n) for s0, n in chunks]
    return np.ascontiguousarray(np.concatenate(blocks, axis=1))


def _pack_w13(wT):
    """[D, H] -> [128, D_T*H], PACKS13-ordered, pack-contiguous."""
    arr = wT.reshape(D_T, P, H).transpose(1, 0, 2)            # [128, D_T, H]
    blocks = [
        arr[:, :, h0 * P:(h0 + hc) * P].reshape(P, D_T * hc * P)
        for h0, hc in PACKS13
    ]
    return np.ascontiguousarray(np.concatenate(blocks, axis=1))


def _pack_w2(w2T):
    """[H, D] -> [2, 128, W2Q*D] (per hidden-slab pack)."""
    npack = H_T // W2Q
    arr = w2T.reshape(npack, W2Q, P, D).transpose(0, 2, 1, 3)
    return np.ascontiguousarray(arr.reshape(npack, P, W2Q * D))


def _build_nc(cap, act="silu"):
    import concourse.mybir as mybir
    import concourse.tile as tile
    from concourse import bacc

    bf16 = mybir.dt.bfloat16
    f32 = mybir.dt.float32
    # "sigmoid" exists only for CoreSim smoke tests (sim lacks Silu)
    Silu = (
        mybir.ActivationFunctionType.Silu
        if act == "silu"
        else mybir.ActivationFunctionType.Sigmoid
    )

    chunks = _chunk_sizes(cap)

    nc = bacc.Bacc()
    xT_d = nc.declare_dram_parameter("xT", [P, D_T * cap], bf16, isOutput=False)
    w1T_d = nc.declare_dram_parameter("w1T", [P, D_T * H], bf16, isOutput=False)
    w3T_d = nc.declare_dram_parameter("w3T", [P, D_T * H], bf16, isOutput=False)
    w2T_d = nc.declare_dram_parameter("w2T", [H_T // W2Q, P, W2Q * D], bf16, isOutput=False)
    yT_d = nc.declare_dram_parameter("yT", [D, cap], f32, isOutput=True)

    with tile.TileContext(nc) as tc:
        with (
            tc.tile_pool(name="wpool", bufs=1) as wpool,
            tc.tile_pool(name="xpool", bufs=2) as xpool,
            tc.tile_pool(name="hpool", bufs=2) as hpool,
            tc.tile_pool(name="gpool", bufs=4) as gpool,
            tc.tile_pool(name="opool", bufs=4) as opool,
            tc.tile_pool(name="pspool", bufs=2, space="PSUM") as pspool,
        ):
            # Every load is one fully-contiguous DMA on the SP HWDGE ring, in
            # exact consumption order (x0, then w1/w3 packs interleaved, then
            # w2). Outputs use the ACT ring so they never queue behind loads.
            def load_x(ci):
                s0, n = chunks[ci]
                off = D_T * s0
                xt = xpool.tile([P, D_T, n], bf16, tag="x", name="x")
                nc.sync.dma_start(xt[:], xT_d[:, off:off + D_T * n])
                return xt

            # All input loads ride the SP HWDGE ring in consumption order.
            # (Putting loads on the ACT ring measurably hurts: its DMAs share
            # the Scalar sequencer with the Silu activations.) Chunk-0's x is
            # split in two with the first w1 pack sandwiched between, so the
            # first gate group's critical prefix is 768KB instead of 1.25MB.
            n0 = chunks[0][1]
            half = D_T // 2
            xs = xpool.tile([P, D_T, n0], bf16, tag="x", name="x")
            nc.sync.dma_start(xs[:, :half, :], xT_d[:, 0:half * n0])
            # w13_tiles[h] -> (tile, column offset of slab h inside the tile)
            w1s, w3s = {}, {}
            off = 0
            for pi, (h0, hc) in enumerate(PACKS13):
                w = D_T * hc * P
                t1 = wpool.tile([P, D_T, hc * P], bf16, tag=f"w1_{h0}", name=f"w1_{h0}")
                nc.sync.dma_start(t1[:].rearrange("p d c -> p (d c)"), w1T_d[:, off:off + w])
                if pi == 0:
                    nc.sync.dma_start(xs[:, half:, :], xT_d[:, half * n0:D_T * n0])
                t3 = wpool.tile([P, D_T, hc * P], bf16, tag=f"w3_{h0}", name=f"w3_{h0}")
                nc.sync.dma_start(t3[:].rearrange("p d c -> p (d c)"), w3T_d[:, off:off + w])
                for k in range(hc):
                    w1s[h0 + k] = (t1, k * P)
                    w3s[h0 + k] = (t3, k * P)
                off += w
            w2q = []
            for q in range(H_T // W2Q):
                t2 = wpool.tile([P, W2Q, D], bf16, tag=f"w2_{q}", name=f"w2_{q}")
                nc.sync.dma_start(t2[:].rearrange("p d c -> p (d c)"), w2T_d[q])
                w2q.append(t2)

            # HAM warmup: full-width (N=512) matmuls on zeros from right after
            # the framework preamble until the first real chunk's data lands
            # (~12.6us). The PE clock gate (HAM) only un-throttles after ~5us
            # of SUSTAINED high-duty matmul activity -- tiny matmuls don't
            # register as busy -- so the warmup stream must look like the real
            # one. The memset rides GpSimd, whose queue is empty once the
            # start barrier completes (~6.2us).
            warm_sb = gpool.tile([P, NTOK], bf16, tag="warm_sb", name="warm_sb")
            nc.gpsimd.memset(warm_sb[:], 0.0)
            # shares the "py" slots so all 8 PSUM banks go to real tiles
            warm_ps = pspool.tile([P, NTOK], f32, tag="py", name="warm_ps", bufs=3)
            for _ in range(14):
                nc.tensor.matmul(
                    warm_ps[:16, :], lhsT=warm_sb[:, :16], rhs=warm_sb[:],
                    start=True, stop=True,
                )

            for ci, (s0, n) in enumerate(chunks):
                if ci > 0:
                    xs = load_x(ci)  # chunk-0's xs was loaded above

                # stage 1: hT[h] = silu(w1T.T@xT) * (w3T.T@xT)  -> [128, n] bf16
                hts = []
                for h in range(H_T):
                    t1, c0 = w1s[h]
                    t3, c3 = w3s[h]
                    pg = pspool.tile([P, NTOK], f32, tag="pg", name="pg")
                    for d in range(D_T):
                        nc.tensor.matmul(
                            pg[:, :n],
                            lhsT=t1[:, d, c0:c0 + P],
                            rhs=xs[:, d, :],
                            start=(d == 0),
                            stop=(d == D_T - 1),
                        )
                    pu = pspool.tile([P, NTOK], f32, tag="pu", name="pu", bufs=3)
                    for d in range(D_T):
                        nc.tensor.matmul(
                            pu[:, :n],
                            lhsT=t3[:, d, c3:c3 + P],
                            rhs=xs[:, d, :],
                            start=(d == 0),
                            stop=(d == D_T - 1),
                        )
                    g = gpool.tile([P, NTOK], bf16, tag="g", name="g")
                    nc.scalar.activation(g[:, :n], pg[:, :n], Silu)
                    ht = hpool.tile([P, NTOK], bf16, tag=f"h_{h}", name=f"h_{h}")
                    nc.vector.tensor_mul(out=ht[:, :n], in0=g[:, :n], in1=pu[:, :n])
                    hts.append(ht)

                # stage 2: yT[do] = sum_h w2T[h,do].T @ hT[h]  -> [128, n] f32
                for do in range(D_T):
                    py = pspool.tile([P, NTOK], f32, tag="py", name="py", bufs=3)
                    for h in range(H_T):
                        nc.tensor.matmul(
                            py[:, :n],
                            lhsT=w2q[h // W2Q][:, h % W2Q, do * P:(do + 1) * P],
                            rhs=hts[h][:, :n],
                            start=(h == 0),
                            stop=(h == H_T - 1),
                        )
                    ot = opool.tile([P, NTOK], f32, tag="o", name="o")
                    nc.vector.tensor_copy(ot[:, :n], py[:, :n])
                    # output DMAs ride the ACT HWDGE ring so they never queue
                    # ahead of input loads on the SP ring; the last chunk's
                    # ride the by-then-idle SP ring to shorten the tail
                    ring = nc.sync if ci == len(chunks) - 1 else nc.scalar
                    ring.dma_start(yT_d[do * P:(do + 1) * P, s0:s0 + n], ot[:, :n])

    nc.finalize()
    return nc


def kernel(x, router_w, w1, w2, w3):
    global LAST_RESULTS
    from concourse.bass_utils import run_bass_kernel_spmd

    x = np.ascontiguousarray(np.asarray(x, dtype=np.float32))
    router_w = np.asarray(router_w, dtype=np.float32)
    flat = x.reshape(T, D)

    # ---- host router (fp32, matches reference math) ----
    logits = flat @ router_w.T                      # [T, E]
    rows = np.arange(T)
    i1 = np.argmax(logits, axis=1)
    v1 = logits[rows, i1]
    masked = logits.copy()
    masked[rows, i1] = -np.inf
    i2 = np.argmax(masked, axis=1)
    v2 = masked[rows, i2]
    # softmax over the two selected logits (v1 >= v2)
    e2 = np.exp(v2 - v1)
    wt1 = 1.0 / (1.0 + e2)
    wt2 = e2 / (1.0 + e2)

    # ---- dispatch: token lists per expert ----
    idxs, wts = [], []
    for e in range(E):
        m1 = i1 == e
        m2 = i2 == e
        idx = np.nonzero(m1 | m2)[0]
        w = np.where(m1[idx], wt1[idx], wt2[idx]).astype(np.float32)
        idxs.append(idx)
        wts.append(w)
    max_cnt = max(len(i) for i in idxs)
    cap = max(NTOK, -(-max_cnt // 8) * 8)
    chunks = _chunk_sizes(cap)

    if cap not in _cache:
        _cache[cap] = _build_nc(cap)
    nc = _cache[cap]

    # ---- per-core inputs (bf16, pre-transposed, pre-packed) ----
    bf = ml_dtypes.bfloat16
    in_maps = []
    for e in range(E):
        idx = idxs[e]
        xTe = np.zeros((D, cap), dtype=bf)
        xTe[:, :len(idx)] = flat[idx].T.astype(bf)
        in_maps.append({
            "xT": _pack_x(xTe, chunks),
            "w1T": _pack_w13(np.ascontiguousarray(w1[e].T).astype(bf)),
            "w3T": _pack_w13(np.ascontiguousarray(w3[e].T).astype(bf)),
            "w2T": _pack_w2(np.ascontiguousarray(w2[e].T).astype(bf)),
        })

    trace = os.environ.get("KERNEL_TRACE", "0") == "1"
    kwargs = {}
    if trace:
        kwargs = dict(trace=True, trace_cores=list(range(E)))
    res = run_bass_kernel_spmd(nc, in_maps, core_ids=list(range(E)), **kwargs)
    LAST_RESULTS = res

    # ---- combine (the "all-to-all" return + weighted sum) ----
    out = np.zeros((T, D), dtype=np.float32)
    for e in range(E):
        idx = idxs[e]
        yT = res.results[e]["yT"]                   # [D, cap] f32
        out[idx] += wts[e][:, None] * yT[:, :len(idx)].T
    return out.reshape(B, S, D)



# revision 7
# speedup vs baseline: 1.4176x; 1.4176x over previous
"""MoE layer (E=8, top-2, SwiGLU experts) on 8 trn2 NeuronCores.

Strategy (expert parallel, host-routed):
  - Router (flat @ router_w.T, top-2, softmax) is computed on host in fp32;
    it is tiny (33 MFLOP) and must match the reference's expert selection
    exactly (min top2-vs-3rd logit gap on these inputs is ~1e-4, far above
    fp32 matmul noise ~1e-6).
  - Tokens are dispatched to core e = expert e (the "all-to-all"), padded to
    a fixed capacity CAP. Each core runs a dense bf16 SwiGLU FFN for its
    expert over its routed tokens: yT = w2T.T @ (silu(w1T.T@xT) * (w3T.T@xT)).
    All tensors are pre-transposed AND pre-packed on host into the exact
    SBUF-resident layouts (partition-major, pack-contiguous) so every device
    DMA is a pure linear copy.
  - Host combines: out[tok] += combine_weight * y (each token appears in
    exactly 2 experts' outputs).

Compute dtype bf16 (PE runs fp32 at 1/4 rate), fp32 PSUM accumulation,
fp32 output.
"""

import os
import numpy as np
import ml_dtypes

B, S, D, H, E = 2, 2048, 1024, 2048, 8
T = B * S
TOP_K = 2
P = 128
NTOK = 512    # max token chunk (matmul free dim / one PSUM bank of fp32)
D_T = D // P  # 8 contraction slabs for stage 1 / output slabs for stage 2
H_T = H // P  # 16 hidden slabs
# w1/w3 stream in hidden-column packs: (h_start, h_count). The first packs
# are single slabs so the first gate group's critical DMA prefix is small.
PACKS13 = [(0, 1), (1, 1)] + [(2 + 2 * i, 2) for i in range(7)]
W2Q = 8       # w2 pack = 8 hidden slabs -> 2 packs

_cache = {}

# set by the last kernel() call when tracing is enabled (KERNEL_TRACE=1)
LAST_RESULTS = None


def _chunk_sizes(cap):
    """First chunk as large as possible (its stage 1 overlaps the weight
    stream, and a larger free dim slows per-h weight consumption below the
    DMA ring bandwidth); remainder split equally (multiples of 8)."""
    first = min(NTOK, cap)
    sizes = [first]
    rem = cap - first
    if rem:
        k = -(-rem // NTOK)
        base, r8 = divmod(rem // 8, k)
        sizes += [(base + (1 if i < r8 else 0)) * 8 for i in range(k)]
    chunks, s = [], 0
    for n in sizes:
        chunks.append((s, n))
        s += n
    return chunks


def _pack_x(xTe, chunks):
    """[D, cap] -> [128, D_T*cap], chunk-blocked, partition-major."""
    arr = xTe.reshape(D_T, P, -1).transpose(1, 0, 2)  # [128, D_T, cap]
    blocks = [arr[:, :, s0:s0 + n].reshape(P, D_T * n) for s0, n in chunks]
    return np.ascontiguousarray(np.concatenate(blocks, axis=1))


def _pack_w13(wT):
    """[D, H] -> [128, D_T*H], PACKS13-ordered, pack-contiguous."""
    arr = wT.reshape(D_T, P, H).transpose(1, 0, 2)            # [128, D_T, H]
    blocks = [
        arr[:, :, h0 * P:(h0 + hc) * P].reshape(P, D_T * hc * P)
        for h0, hc in PACKS13
    ]
    return np.ascontiguousarray(np.concatenate(blocks, axis=1))


def _pack_w2(w2T):
    """[H, D] -> [2, 128, W2Q*D] (per hidden-slab pack)."""
    npack = H_T // W2Q
    arr = w2T.reshape(npack, W2Q, P, D).transpose(0, 2, 1, 3)
    return np.ascontiguousarray(arr.reshape(npack, P, W2Q * D))


def _build_nc(cap, act="silu"):
    import concourse.mybir as mybir
    import concourse.tile as tile
    from concourse import bacc

    bf16 = mybir.dt.bfloat16
    f32 = mybir.dt.float32
    # "sigmoid" exists only for CoreSim smoke tests (sim lacks Silu)
    Silu = (
        mybir.ActivationFunctionType.Silu
        if act == "silu"
        else mybir.ActivationFunctionType.Sigmoid
    )

    chunks = _chunk_sizes(cap)

    nc = bacc.Bacc()
    xT_d = nc.declare_dram_parameter("xT", [P, D_T * cap], bf16, isOutput=False)
    w1T_d = nc.declare_dram_parameter("w1T", [P, D_T * H], bf16, isOutput=False)
    w3T_d = nc.declare_dram_parameter("w3T", [P, D_T * H], bf16, isOutput=False)
    w2T_d = nc.declare_dram_parameter("w2T", [H_T // W2Q, P, W2Q * D], bf16, isOutput=False)
    yT_d = nc.declare_dram_parameter("yT", [D, cap], bf16, isOutput=True)

    with tile.TileContext(nc) as tc:
        with (
            tc.tile_pool(name="wpool", bufs=1) as wpool,
            tc.tile_pool(name="xpool", bufs=2) as xpool,
            tc.tile_pool(name="hpool", bufs=2) as hpool,
            tc.tile_pool(name="gpool", bufs=4) as gpool,
            tc.tile_pool(name="opool", bufs=4) as opool,
            tc.tile_pool(name="pspool", bufs=2, space="PSUM") as pspool,
        ):
            # Every load is one fully-contiguous DMA on the SP HWDGE ring, in
            # exact consumption order (x0, then w1/w3 packs interleaved, then
            # w2). Outputs use the ACT ring so they never queue behind loads.
            def load_x(ci):
                s0, n = chunks[ci]
                off = D_T * s0
                xt = xpool.tile([P, D_T, n], bf16, tag="x", name="x")
                nc.sync.dma_start(xt[:], xT_d[:, off:off + D_T * n])
                return xt

            # Input loads ride the SP HWDGE ring in consumption order, EXCEPT
            # the first w1/w3 packs, which ride the (otherwise idle at start)
            # ACT ring concurrently with chunk-0's x on the SP ring: the
            # first gate group's critical prefix becomes max(x0, w1p0+w3p0)
            # instead of their sum. Mid-stream loads stay off the ACT ring
            # (its DMAs share the Scalar sequencer with the Silu work).
            n0 = chunks[0][1]
            half = D_T // 2
            xs = xpool.tile([P, D_T, n0], bf16, tag="x", name="x")
            nc.sync.dma_start(xs[:, :half, :], xT_d[:, 0:half * n0])
            # w13_tiles[h] -> (tile, column offset of slab h inside the tile)
            w1s, w3s = {}, {}
            off = 0
            for pi, (h0, hc) in enumerate(PACKS13):
                w = D_T * hc * P
                wring = nc.scalar if pi == 0 else nc.sync
                t1 = wpool.tile([P, D_T, hc * P], bf16, tag=f"w1_{h0}", name=f"w1_{h0}")
                wring.dma_start(t1[:].rearrange("p d c -> p (d c)"), w1T_d[:, off:off + w])
                if pi == 0:
                    nc.sync.dma_start(xs[:, half:, :], xT_d[:, half * n0:D_T * n0])
                t3 = wpool.tile([P, D_T, hc * P], bf16, tag=f"w3_{h0}", name=f"w3_{h0}")
                wring.dma_start(t3[:].rearrange("p d c -> p (d c)"), w3T_d[:, off:off + w])
                for k in range(hc):
                    w1s[h0 + k] = (t1, k * P)
                    w3s[h0 + k] = (t3, k * P)
                off += w
            w2q = []
            for q in range(H_T // W2Q):
                t2 = wpool.tile([P, W2Q, D], bf16, tag=f"w2_{q}", name=f"w2_{q}")
                nc.sync.dma_start(t2[:].rearrange("p d c -> p (d c)"), w2T_d[q])
                w2q.append(t2)

            # HAM warmup: full-width (N=512) matmuls on zeros from right after
            # the framework preamble until the first real chunk's data lands
            # (~12.6us). The PE clock gate (HAM) only un-throttles after ~5us
            # of SUSTAINED high-duty matmul activity -- tiny matmuls don't
            # register as busy -- so the warmup stream must look like the real
            # one. The memset rides GpSimd, whose queue is empty once the
            # start barrier completes (~6.2us).
            warm_sb = gpool.tile([P, NTOK], bf16, tag="warm_sb", name="warm_sb")
            nc.gpsimd.memset(warm_sb[:], 0.0)
            # shares the "py" slots so all 8 PSUM banks go to real tiles
            warm_ps = pspool.tile([P, NTOK], f32, tag="py", name="warm_ps", bufs=3)
            for _ in range(14):
                nc.tensor.matmul(
                    warm_ps[:16, :], lhsT=warm_sb[:, :16], rhs=warm_sb[:],
                    start=True, stop=True,
                )

            for ci, (s0, n) in enumerate(chunks):
                if ci > 0:
                    xs = load_x(ci)  # chunk-0's xs was loaded above

                # stage 1: hT[h] = silu(w1T.T@xT) * (w3T.T@xT)  -> [128, n] bf16
                hts = []
                for h in range(H_T):
                    t1, c0 = w1s[h]
                    t3, c3 = w3s[h]
                    pg = pspool.tile([P, NTOK], f32, tag="pg", name="pg")
                    for d in range(D_T):
                        nc.tensor.matmul(
                            pg[:, :n],
                            lhsT=t1[:, d, c0:c0 + P],
                            rhs=xs[:, d, :],
                            start=(d == 0),
                            stop=(d == D_T - 1),
                        )
                    pu = pspool.tile([P, NTOK], f32, tag="pu", name="pu", bufs=3)
                    for d in range(D_T):
                        nc.tensor.matmul(
                            pu[:, :n],
                            lhsT=t3[:, d, c3:c3 + P],
                            rhs=xs[:, d, :],
                            start=(d == 0),
                            stop=(d == D_T - 1),
                        )
                    g = gpool.tile([P, NTOK], bf16, tag="g", name="g")
                    nc.scalar.activation(g[:, :n], pg[:, :n], Silu)
                    ht = hpool.tile([P, NTOK], bf16, tag=f"h_{h}", name=f"h_{h}")
                    nc.vector.tensor_mul(out=ht[:, :n], in0=g[:, :n], in1=pu[:, :n])
                    hts.append(ht)

                # stage 2: yT[do] = sum_h w2T[h,do].T @ hT[h]  -> [128, n] f32
                for do in range(D_T):
                    py = pspool.tile([P, NTOK], f32, tag="py", name="py", bufs=3)
                    for h in range(H_T):
                        nc.tensor.matmul(
                            py[:, :n],
                            lhsT=w2q[h // W2Q][:, h % W2Q, do * P:(do + 1) * P],
                            rhs=hts[h][:, :n],
                            start=(h == 0),
                            stop=(h == H_T - 1),
                        )
                    ot = opool.tile([P, NTOK], bf16, tag="o", name="o")
                    nc.vector.tensor_copy(ot[:, :n], py[:, :n])
                    # output DMAs ride the ACT HWDGE ring so they never queue
                    # ahead of input loads on the SP ring; the last chunk's
                    # ride the by-then-idle SP ring to shorten the tail
                    ring = nc.sync if ci == len(chunks) - 1 else nc.scalar
                    ring.dma_start(yT_d[do * P:(do + 1) * P, s0:s0 + n], ot[:, :n])

    nc.finalize()
    return nc


def kernel(x, router_w, w1, w2, w3):
    global LAST_RESULTS
    from concourse.bass_utils import run_bass_kernel_spmd

    x = np.ascontiguousarray(np.asarray(x, dtype=np.float32))
    router_w = np.asarray(router_w, dtype=np.float32)
    flat = x.reshape(T, D)

    # ---- host router (fp32, matches reference math) ----
    logits = flat @ router_w.T                      # [T, E]
    rows = np.arange(T)
    i1 = np.argmax(logits, axis=1)
    v1 = logits[rows, i1]
    masked = logits.copy()
    masked[rows, i1] = -np.inf
    i2 = np.argmax(masked, axis=1)
    v2 = masked[rows, i2]
    # softmax over the two selected logits (v1 >= v2)
    e2 = np.exp(v2 - v1)
    wt1 = 1.0 / (1.0 + e2)
    wt2 = e2 / (1.0 + e2)

    # ---- dispatch: token lists per expert ----
    idxs, wts = [], []
    for e in range(E):
        m1 = i1 == e
        m2 = i2 == e
        idx = np.nonzero(m1 | m2)[0]
        w = np.where(m1[idx], wt1[idx], wt2[idx]).astype(np.float32)
        idxs.append(idx)
        wts.append(w)
    max_cnt = max(len(i) for i in idxs)
    cap = max(NTOK, -(-max_cnt // 8) * 8)
    chunks = _chunk_sizes(cap)

    if cap not in _cache:
        _cache[cap] = _build_nc(cap)
    nc = _cache[cap]

    # ---- per-core inputs (bf16, pre-transposed, pre-packed) ----
    bf = ml_dtypes.bfloat16
    in_maps = []
    for e in range(E):
        idx = idxs[e]
        xTe = np.zeros((D, cap), dtype=bf)
        xTe[:, :len(idx)] = flat[idx].T.astype(bf)
        in_maps.append({
            "xT": _pack_x(xTe, chunks),
            "w1T": _pack_w13(np.ascontiguousarray(w1[e].T).astype(bf)),
            "w3T": _pack_w13(np.ascontiguousarray(w3[e].T).astype(bf)),
            "w2T": _pack_w2(np.ascontiguousarray(w2[e].T).astype(bf)),
        })

    trace = os.environ.get("KERNEL_TRACE", "0") == "1"
    kwargs = {}
    if trace:
        kwargs = dict(trace=True, trace_cores=list(range(E)))
    res = run_bass_kernel_spmd(nc, in_maps, core_ids=list(range(E)), **kwargs)
    LAST_RESULTS = res

    # ---- combine (the "all-to-all" return + weighted sum) ----
    out = np.zeros((T, D), dtype=np.float32)
    for e in range(E):
        idx = idxs[e]
        yT = res.results[e]["yT"].astype(np.float32)   # [D, cap]
        out[idx] += wts[e][:, None] * yT[:, :len(idx)].T
    return out.reshape(B, S, D)



# revision 8
# speedup vs baseline: 1.4434x; 1.0182x over previous
"""MoE layer (E=8, top-2, SwiGLU experts) on 8 trn2 NeuronCores.

Strategy (expert parallel, host-routed):
  - Router (flat @ router_w.T, top-2, softmax) is computed on host in fp32;
    it is tiny (33 MFLOP) and must match the reference's expert selection
    exactly (min top2-vs-3rd logit gap on these inputs is ~1e-4, far above
    fp32 matmul noise ~1e-6).
  - Tokens are dispatched to core e = expert e (the "all-to-all"), padded to
    a fixed capacity CAP. Each core runs a dense bf16 SwiGLU FFN for its
    expert over its routed tokens: yT = w2T.T @ (silu(w1T.T@xT) * (w3T.T@xT)).
    All tensors are pre-transposed AND pre-packed on host into the exact
    SBUF-resident layouts (partition-major, pack-contiguous) so every device
    DMA is a pure linear copy.
  - Host combines: out[tok] += combine_weight * y (each token appears in
    exactly 2 experts' outputs).

Compute dtype bf16 (PE runs fp32 at 1/4 rate), fp32 PSUM accumulation,
fp32 output.
"""

import os
import numpy as np
import ml_dtypes

B, S, D, H, E = 2, 2048, 1024, 2048, 8
T = B * S
TOP_K = 2
P = 128
NTOK = 512    # max token chunk (matmul free dim / one PSUM bank of fp32)
D_T = D // P  # 8 contraction slabs for stage 1 / output slabs for stage 2
H_T = H // P  # 16 hidden slabs
# w1/w3 stream in hidden-column packs: (h_start, h_count). The first packs
# are single slabs so the first gate group's critical DMA prefix is small.
PACKS13 = [(0, 1), (1, 1)] + [(2 + 2 * i, 2) for i in range(7)]
W2Q = 8       # w2 pack = 8 hidden slabs -> 2 packs

_cache = {}

# set by the last kernel() call when tracing is enabled (KERNEL_TRACE=1)
LAST_RESULTS = None


def _chunk_sizes(cap):
    """First chunk as large as possible (its stage 1 overlaps the weight
    stream, and a larger free dim slows per-h weight consumption below the
    DMA ring bandwidth); remainder split equally (multiples of 8)."""
    first = min(NTOK, cap)
    sizes = [first]
    rem = cap - first
    if rem:
        k = -(-rem // NTOK)
        base, r8 = divmod(rem // 8, k)
        sizes += [(base + (1 if i < r8 else 0)) * 8 for i in range(k)]
    chunks, s = [], 0
    for n in sizes:
        chunks.append((s, n))
        s += n
    return chunks


def _pack_x(xTe, chunks):
    """[D, cap] -> [128, D_T*cap], chunk-blocked, partition-major."""
    arr = xTe.reshape(D_T, P, -1).transpose(1, 0, 2)  # [128, D_T, cap]
    blocks = [arr[:, :, s0:s0 + n].reshape(P, D_T * n) for s0, n in chunks]
    return np.ascontiguousarray(np.concatenate(blocks, axis=1))


def _pack_w13(wT):
    """[D, H] -> [128, D_T*H], PACKS13-ordered, pack-contiguous."""
    arr = wT.reshape(D_T, P, H).transpose(1, 0, 2)            # [128, D_T, H]
    blocks = [
        arr[:, :, h0 * P:(h0 + hc) * P].reshape(P, D_T * hc * P)
        for h0, hc in PACKS13
    ]
    return np.ascontiguousarray(np.concatenate(blocks, axis=1))


def _pack_w2(w2T):
    """[H, D] -> [2, 128, W2Q*D] (per hidden-slab pack)."""
    npack = H_T // W2Q
    arr = w2T.reshape(npack, W2Q, P, D).transpose(0, 2, 1, 3)
    return np.ascontiguousarray(arr.reshape(npack, P, W2Q * D))


def _build_nc(cap, act="silu"):
    import concourse.mybir as mybir
    import concourse.tile as tile
    from concourse import bacc

    bf16 = mybir.dt.bfloat16
    f32 = mybir.dt.float32
    # "sigmoid" exists only for CoreSim smoke tests (sim lacks Silu)
    Silu = (
        mybir.ActivationFunctionType.Silu
        if act == "silu"
        else mybir.ActivationFunctionType.Sigmoid
    )

    chunks = _chunk_sizes(cap)

    nc = bacc.Bacc()
    xT_d = nc.declare_dram_parameter("xT", [P, D_T * cap], bf16, isOutput=False)
    w1T_d = nc.declare_dram_parameter("w1T", [P, D_T * H], bf16, isOutput=False)
    w3T_d = nc.declare_dram_parameter("w3T", [P, D_T * H], bf16, isOutput=False)
    w2T_d = nc.declare_dram_parameter("w2T", [H_T // W2Q, P, W2Q * D], bf16, isOutput=False)
    yT_d = nc.declare_dram_parameter("yT", [D, cap], bf16, isOutput=True)

    with tile.TileContext(nc) as tc:
        with (
            tc.tile_pool(name="wpool", bufs=1) as wpool,
            tc.tile_pool(name="xpool", bufs=2) as xpool,
            tc.tile_pool(name="hpool", bufs=2) as hpool,
            tc.tile_pool(name="gpool", bufs=4) as gpool,
            tc.tile_pool(name="opool", bufs=4) as opool,
            tc.tile_pool(name="pspool", bufs=2, space="PSUM") as pspool,
        ):
            # Every load is one fully-contiguous DMA on the SP HWDGE ring, in
            # exact consumption order (x0, then w1/w3 packs interleaved, then
            # w2). Outputs use the ACT ring so they never queue behind loads.
            def load_x(ci):
                s0, n = chunks[ci]
                off = D_T * s0
                xt = xpool.tile([P, D_T, n], bf16, tag="x", name="x")
                nc.sync.dma_start(xt[:], xT_d[:, off:off + D_T * n])
                return xt

            # Input loads ride the SP HWDGE ring in consumption order, EXCEPT
            # the first w1/w3 packs, which ride the (otherwise idle at start)
            # ACT ring concurrently with chunk-0's x on the SP ring: the
            # first gate group's critical prefix becomes max(x0, w1p0+w3p0)
            # instead of their sum. Mid-stream loads stay off the ACT ring
            # (its DMAs share the Scalar sequencer with the Silu work).
            n0 = chunks[0][1]
            half = D_T // 2
            xs = xpool.tile([P, D_T, n0], bf16, tag="x", name="x")
            nc.sync.dma_start(xs[:, :half, :], xT_d[:, 0:half * n0])
            # w13_tiles[h] -> (tile, column offset of slab h inside the tile)
            w1s, w3s = {}, {}
            off = 0
            for pi, (h0, hc) in enumerate(PACKS13):
                w = D_T * hc * P
                wring = nc.scalar if pi == 0 else nc.sync
                t1 = wpool.tile([P, D_T, hc * P], bf16, tag=f"w1_{h0}", name=f"w1_{h0}")
                wring.dma_start(t1[:].rearrange("p d c -> p (d c)"), w1T_d[:, off:off + w])
                if pi == 0:
                    nc.sync.dma_start(xs[:, half:, :], xT_d[:, half * n0:D_T * n0])
                t3 = wpool.tile([P, D_T, hc * P], bf16, tag=f"w3_{h0}", name=f"w3_{h0}")
                wring.dma_start(t3[:].rearrange("p d c -> p (d c)"), w3T_d[:, off:off + w])
                for k in range(hc):
                    w1s[h0 + k] = (t1, k * P)
                    w3s[h0 + k] = (t3, k * P)
                off += w
            w2q = []
            for q in range(H_T // W2Q):
                t2 = wpool.tile([P, W2Q, D], bf16, tag=f"w2_{q}", name=f"w2_{q}")
                nc.sync.dma_start(t2[:].rearrange("p d c -> p (d c)"), w2T_d[q])
                w2q.append(t2)

            # HAM warmup: full-width (N=512) matmuls on zeros from right after
            # the framework preamble until the first real chunk's data lands
            # (~12.6us). The PE clock gate (HAM) only un-throttles after ~5us
            # of SUSTAINED high-duty matmul activity -- tiny matmuls don't
            # register as busy -- so the warmup stream must look like the real
            # one. The memset rides GpSimd, whose queue is empty once the
            # start barrier completes (~6.2us).
            warm_sb = gpool.tile([P, NTOK], bf16, tag="warm_sb", name="warm_sb")
            nc.gpsimd.memset(warm_sb[:], 0.0)
            # shares the "py" slots so all 8 PSUM banks go to real tiles
            warm_ps = pspool.tile([P, NTOK], f32, tag="py", name="warm_ps", bufs=3)
            for _ in range(14):
                nc.tensor.matmul(
                    warm_ps[:16, :], lhsT=warm_sb[:, :16], rhs=warm_sb[:],
                    start=True, stop=True,
                )

            for ci, (s0, n) in enumerate(chunks):
                if ci > 0:
                    xs = load_x(ci)  # chunk-0's xs was loaded above

                # stage 1: hT[h] = silu(w1T.T@xT) * (w3T.T@xT)  -> [128, n] bf16
                hts = []
                for h in range(H_T):
                    t1, c0 = w1s[h]
                    t3, c3 = w3s[h]
                    pg = pspool.tile([P, NTOK], f32, tag="pg", name="pg")
                    for d in range(D_T):
                        nc.tensor.matmul(
                            pg[:, :n],
                            lhsT=t1[:, d, c0:c0 + P],
                            rhs=xs[:, d, :],
                            start=(d == 0),
                            stop=(d == D_T - 1),
                        )
                    pu = pspool.tile([P, NTOK], f32, tag="pu", name="pu", bufs=3)
                    for d in range(D_T):
                        nc.tensor.matmul(
                            pu[:, :n],
                            lhsT=t3[:, d, c3:c3 + P],
                            rhs=xs[:, d, :],
                            start=(d == 0),
                            stop=(d == D_T - 1),
                        )
                    g = gpool.tile([P, NTOK], bf16, tag="g", name="g")
                    nc.scalar.activation(g[:, :n], pg[:, :n], Silu)
                    ht = hpool.tile([P, NTOK], bf16, tag=f"h_{h}", name=f"h_{h}")
                    nc.vector.tensor_mul(out=ht[:, :n], in0=g[:, :n], in1=pu[:, :n])
                    hts.append(ht)

                # stage 2: yT[do] = sum_h w2T[h,do].T @ hT[h]  -> [128, n] f32
                for do in range(D_T):
                    py = pspool.tile([P, NTOK], f32, tag="py", name="py", bufs=3)
                    for h in range(H_T):
                        nc.tensor.matmul(
                            py[:, :n],
                            lhsT=w2q[h // W2Q][:, h % W2Q, do * P:(do + 1) * P],
                            rhs=hts[h][:, :n],
                            start=(h == 0),
                            stop=(h == H_T - 1),
                        )
                    # bufs=8: a full chunk of stage-2 copies can land without
                    # waiting on any output-DMA completion, so a transient
                    # ACT-ring stall never backpressures the PE via py tiles.
                    ot = opool.tile([P, NTOK], bf16, tag="o", name="o", bufs=8)
                    nc.vector.tensor_copy(ot[:, :n], py[:, :n])
                    # output DMAs ride the ACT HWDGE ring so they never queue
                    # ahead of input loads on the SP ring; the last chunk's
                    # ride the by-then-idle SP ring to shorten the tail
                    ring = nc.sync if ci == len(chunks) - 1 else nc.scalar
                    ring.dma_start(yT_d[do * P:(do + 1) * P, s0:s0 + n], ot[:, :n])

    nc.finalize()
    return nc


def kernel(x, router_w, w1, w2, w3):
    global LAST_RESULTS
    from concourse.bass_utils import run_bass_kernel_spmd

    x = np.ascontiguousarray(np.asarray(x, dtype=np.float32))
    router_w = np.asarray(router_w, dtype=np.float32)
    flat = x.reshape(T, D)

    # ---- host router (fp32, matches reference math) ----
    logits = flat @ router_w.T                      # [T, E]
    rows = np.arange(T)
    i1 = np.argmax(logits, axis=1)
    v1 = logits[rows, i1]
    masked = logits.copy()
    masked[rows, i1] = -np.inf
    i2 = np.argmax(masked, axis=1)
    v2 = masked[rows, i2]
    # softmax over the two selected logits (v1 >= v2)
    e2 = np.exp(v2 - v1)
    wt1 = 1.0 / (1.0 + e2)
    wt2 = e2 / (1.0 + e2)

    # ---- dispatch: token lists per expert ----
    idxs, wts = [], []
    for e in range(E):
        m1 = i1 == e
        m2 = i2 == e
        idx = np.nonzero(m1 | m2)[0]
        w = np.where(m1[idx], wt1[idx], wt2[idx]).astype(np.float32)
        idxs.append(idx)
        wts.append(w)
    max_cnt = max(len(i) for i in idxs)
    cap = max(NTOK, -(-max_cnt // 8) * 8)
    chunks = _chunk_sizes(cap)

    if cap not in _cache:
        _cache[cap] = _build_nc(cap)
    nc = _cache[cap]

    # ---- per-core inputs (bf16, pre-transposed, pre-packed) ----
    bf = ml_dtypes.bfloat16
    in_maps = []
    for e in range(E):
        idx = idxs[e]
        xTe = np.zeros((D, cap), dtype=bf)
        xTe[:, :len(idx)] = flat[idx].T.astype(bf)
        in_maps.append({
            "xT": _pack_x(xTe, chunks),
            "w1T": _pack_w13(np.ascontiguousarray(w1[e].T).astype(bf)),
            "w3T": _pack_w13(np.ascontiguousarray(w3[e].T).astype(bf)),
            "w2T": _pack_w2(np.ascontiguousarray(w2[e].T).astype(bf)),
        })

    trace = os.environ.get("KERNEL_TRACE", "0") == "1"
    kwargs = {}
    if trace:
        kwargs = dict(trace=True, trace_cores=list(range(E)))
    res = run_bass_kernel_spmd(nc, in_maps, core_ids=list(range(E)), **kwargs)
    LAST_RESULTS = res

    # ---- combine (the "all-to-all" return + weighted sum) ----
    out = np.zeros((T, D), dtype=np.float32)
    for e in range(E):
        idx = idxs[e]
        yT = res.results[e]["yT"].astype(np.float32)   # [D, cap]
        out[idx] += wts[e][:, None] * yT[:, :len(idx)].T
    return out.reshape(B, S, D)



# revision 9
# speedup vs baseline: 1.4476x; 1.0029x over previous
"""MoE layer (E=8, top-2, SwiGLU experts) on 8 trn2 NeuronCores.

Strategy (expert-pair x tensor-parallel hybrid, host-routed):
  - Router on host in fp32 (exactly matches the reference's selection).
  - The 8 experts are paired big-with-small by routed-token count; each of
    the 4 pairs gets 2 cores. Core (pair p, half h) processes BOTH experts
    of its pair against its OWN 1024-row hidden half:
        g = w1h.T @ x; u = w3h.T @ x; h = silu(g)*u; y_partial = w2h.T @ h
    and the host sums the two halves' partials. Every core runs the same
    program over [C1 | C2] token groups (C1 = max big-expert count, C2 =
    max small-expert count over pairs), so per-core work = (C1+C2)/2 full-H
    token-pairs ~= 1052 -- below the 1080 an expert-per-core split pays for
    the most-loaded expert -- while x is only duplicated 2x (per-core DMA
    ~21 MB, same regime as expert-parallel).
  - All tensors pre-transposed and pre-packed on host so every device DMA
    is a pure linear copy.

Compute bf16 (fp32 PSUM accumulation); output partials bf16 (adds ~0.1%
rel err, negligible vs the 2e-2 gate).
"""

import os
import numpy as np
import ml_dtypes

B, S, D, H, E = 2, 2048, 1024, 2048, 8
T = B * S
TOP_K = 2
P = 128
NTOK = 512    # max matmul free dim / one PSUM bank of fp32
D_T = D // P  # 8 contraction slabs for stage 1 / output slabs for stage 2
TP = 2        # hidden-dim shards per expert pair
HS = H // TP  # per-core hidden slice
HS_T = HS // P  # 8 hidden slabs per expert slot
NPAIR = E // 2

_cache = {}

# set by the last kernel() call when tracing is enabled (KERNEL_TRACE=1)
LAST_RESULTS = None


def _chunk_sizes(ge):
    """First chunk as large as possible (a longer chunk 0 spreads the
    expert's 6.3MB weight-set consumption over more compute, keeping demand
    under the DMA ring bandwidth); remainder split evenly in multiples of 8.
    Every chunk stays >=256 for ge>=1024 so the free dim hides LDWEIGHTS."""
    first = min(NTOK, ge)
    sizes = [first]
    rem = ge - first
    if rem:
        k = -(-rem // NTOK)
        base, r8 = divmod(rem // 8, k)
        sizes += [(base + (1 if i < r8 else 0)) * 8 for i in range(k)]
    return sizes


def _pack_x(xT, chunks):
    """[D, cap] -> [128, D_T*cap], chunk-blocked, partition-major."""
    arr = xT.reshape(D_T, P, -1).transpose(1, 0, 2)   # [128, D_T, cap]
    blocks = []
    s0 = 0
    for n in chunks:
        blocks.append(arr[:, :, s0:s0 + n].reshape(P, D_T * n))
        s0 += n
    return np.ascontiguousarray(np.concatenate(blocks, axis=1))


def _build_nc(caps, act="silu"):
    """caps = (C1, C2): the two per-core group capacities."""
    import concourse.mybir as mybir
    import concourse.tile as tile
    from concourse import bacc

    bf16 = mybir.dt.bfloat16
    f32 = mybir.dt.float32
    Silu = (
        mybir.ActivationFunctionType.Silu
        if act == "silu"
        else mybir.ActivationFunctionType.Sigmoid
    )

    group_chunks = [_chunk_sizes(c) for c in caps]
    capall = sum(caps)

    nc = bacc.Bacc()
    xT_d = nc.declare_dram_parameter("xT", [P, D_T * capall], bf16, isOutput=False)
    # per (slot, h-slab, proj) weight pack, stage-1 consumption order
    w13_d = nc.declare_dram_parameter(
        "w13T", [2 * HS_T * 2, P, D_T * P], bf16, isOutput=False)
    w2_d = nc.declare_dram_parameter("w2T", [2, P, HS_T * D], bf16, isOutput=False)
    yT_d = nc.declare_dram_parameter("yT", [D, capall], bf16, isOutput=True)

    with tile.TileContext(nc) as tc:
        with (
            tc.tile_pool(name="wpool", bufs=1) as wpool,
            tc.tile_pool(name="xpool", bufs=2) as xpool,
            tc.tile_pool(name="hpool", bufs=2) as hpool,
            tc.tile_pool(name="gpool", bufs=4) as gpool,
            tc.tile_pool(name="opool", bufs=4) as opool,
            tc.tile_pool(name="pspool", bufs=2, space="PSUM") as pspool,
        ):
            chunk_list = []   # (slot, col offset, n)
            off = 0
            for slot, ch in enumerate(group_chunks):
                for n in ch:
                    chunk_list.append((slot, off, n))
                    off += n
            n_chunks = len(chunk_list)

            # ALL inputs are loaded up front in consumption order and stay
            # SBUF-resident (no pool recycling): any mid-stream DMA hiccup
            # would stall the PE and -- via a HAM re-throttle -- cost ~2x the
            # stall, so the steady-state stream must depend on no DMA at all
            # (outputs ride the separate ACT ring).
            def load_x(idx):
                slot, s0, n = chunk_list[idx]
                xt = xpool.tile([P, D_T, n], bf16, tag=f"x_{idx}", name=f"x_{idx}")
                nc.sync.dma_start(xt[:], xT_d[:, D_T * s0:D_T * (s0 + n)])
                return xt

            def load_w13_pack(slot, s, pidx, ring=None):
                t = wpool.tile([P, D_T, P], bf16,
                               tag=f"w13_{slot}_{s}_{pidx}",
                               name=f"w13_{slot}_{s}_{pidx}")
                (ring or nc.sync).dma_start(
                    t[:].rearrange("p d c -> p (d c)"),
                    w13_d[slot * HS_T * 2 + s * 2 + pidx])
                return t

            def load_w2(slot):
                t2 = wpool.tile([P, HS_T, D], bf16,
                                tag=f"w2_{slot}", name=f"w2_{slot}")
                nc.sync.dma_start(t2[:].rearrange("p d c -> p (d c)"), w2_d[slot])
                return t2

            # Chunk 0's x is split in two with slot 0's first w1 pack
            # sandwiched between (and the first w3 pack on the ACT ring,
            # idle until outputs begin), keeping the first gate group's
            # critical DMA prefix small.
            n0 = chunk_list[0][2]
            xs0 = xpool.tile([P, D_T, n0], bf16, tag="x_0", name="x_0")
            half = D_T // 2
            nc.sync.dma_start(xs0[:, :half, :], xT_d[:, 0:half * n0])
            w13t = {0: [load_w13_pack(0, 0, 0, ring=nc.scalar)]}
            nc.sync.dma_start(xs0[:, half:, :], xT_d[:, half * n0:D_T * n0])
            w13t[0].append(load_w13_pack(0, 0, 1, ring=nc.scalar))
            for s in range(1, HS_T):
                for pidx in range(2):
                    w13t[0].append(load_w13_pack(0, s, pidx))
            w2t = {0: load_w2(0)}
            xtiles = {0: xs0}
            # rest of slot 0's x, then slot 1's weights, then slot 1's x
            ns0 = len(group_chunks[0])
            for idx in range(1, ns0):
                xtiles[idx] = load_x(idx)
            w13t[1] = [load_w13_pack(1, s, pidx)
                       for s in range(HS_T) for pidx in range(2)]
            w2t[1] = load_w2(1)
            for idx in range(ns0, n_chunks):
                xtiles[idx] = load_x(idx)

            # HAM warmup: full-width (N=512) matmuls on zeros from right
            # after the framework preamble until the first chunk's data
            # lands. The PE clock gate (HAM) only un-throttles after ~3-5us
            # of SUSTAINED high-duty matmul activity -- tiny matmuls don't
            # register as busy -- so the warmup stream must look real.
            warm_sb = gpool.tile([P, NTOK], bf16, tag="warm_sb", name="warm_sb")
            nc.gpsimd.memset(warm_sb[:], 0.0)
            warm_ps = pspool.tile([P, NTOK], f32, tag="py", name="warm_ps", bufs=3)
            for _ in range(14):
                nc.tensor.matmul(
                    warm_ps[:16, :], lhsT=warm_sb[:, :16], rhs=warm_sb[:],
                    start=True, stop=True,
                )

            for idx, (slot, s0, n) in enumerate(chunk_list):
                xs = xtiles[idx]
                w13 = w13t[slot]
                t2 = w2t[slot]

                # stage 1: hT[s] = silu(w1s.T@xT) * (w3s.T@xT)  [128, n] bf16
                hts = []
                for s in range(HS_T):
                    t1 = w13[s * 2]
                    t3 = w13[s * 2 + 1]
                    pg = pspool.tile([P, NTOK], f32, tag="pg", name="pg")
                    for d in range(D_T):
                        nc.tensor.matmul(
                            pg[:, :n], lhsT=t1[:, d, :], rhs=xs[:, d, :],
                            start=(d == 0), stop=(d == D_T - 1),
                        )
                    pu = pspool.tile([P, NTOK], f32, tag="pu", name="pu", bufs=3)
                    for d in range(D_T):
                        nc.tensor.matmul(
                            pu[:, :n], lhsT=t3[:, d, :], rhs=xs[:, d, :],
                            start=(d == 0), stop=(d == D_T - 1),
                        )
                    g = gpool.tile([P, NTOK], bf16, tag="g", name="g")
                    nc.scalar.activation(g[:, :n], pg[:, :n], Silu)
                    ht = hpool.tile([P, NTOK], bf16, tag=f"h_{s}", name=f"h_{s}")
                    nc.vector.tensor_mul(out=ht[:, :n], in0=g[:, :n], in1=pu[:, :n])
                    hts.append(ht)

                # stage 2: yT[do] = sum_s w2s[s,do].T @ hT[s]  [128, n] bf16
                # All outputs ride the ACT ring -- it stays warm from
                # streaming outputs all along, while the SP ring is cold-idle
                # by the last chunk and would pay ~2us of DGE re-spin-up.
                for do in range(D_T):
                    py = pspool.tile([P, NTOK], f32, tag="py", name="py", bufs=3)
                    for s in range(HS_T):
                        nc.tensor.matmul(
                            py[:, :n],
                            lhsT=t2[:, s, do * P:(do + 1) * P],
                            rhs=hts[s][:, :n],
                            start=(s == 0), stop=(s == HS_T - 1),
                        )
                    # bufs=8: the up-front input flood keeps the 16 HW DMA
                    # queues busy for the first ~55us, starving the ACT
                    # ring's output DMAs; a full chunk of copies must be able
                    # to land without waiting on any output-DMA completion,
                    # else the py->copy->ot chain backpressures the PE.
                    ot = opool.tile([P, NTOK], bf16, tag="o", name="o", bufs=8)
                    nc.vector.tensor_copy(ot[:, :n], py[:, :n])
                    nc.scalar.dma_start(yT_d[do * P:(do + 1) * P, s0:s0 + n], ot[:, :n])

    nc.finalize()
    return nc


def kernel(x, router_w, w1, w2, w3):
    global LAST_RESULTS
    from concourse.bass_utils import run_bass_kernel_spmd

    x = np.ascontiguousarray(np.asarray(x, dtype=np.float32))
    router_w = np.asarray(router_w, dtype=np.float32)
    flat = x.reshape(T, D)

    # ---- host router (fp32, matches reference math) ----
    logits = flat @ router_w.T                      # [T, E]
    rows = np.arange(T)
    i1 = np.argmax(logits, axis=1)
    v1 = logits[rows, i1]
    masked = logits.copy()
    masked[rows, i1] = -np.inf
    i2 = np.argmax(masked, axis=1)
    v2 = masked[rows, i2]
    e2 = np.exp(v2 - v1)
    wt1 = 1.0 / (1.0 + e2)
    wt2 = e2 / (1.0 + e2)

    # ---- dispatch: token lists per expert ----
    idxs, wts = [], []
    for e in range(E):
        m1 = i1 == e
        m2 = i2 == e
        idx = np.nonzero(m1 | m2)[0]
        w = np.where(m1[idx], wt1[idx], wt2[idx]).astype(np.float32)
        idxs.append(idx)
        wts.append(w)
    cnts = np.array([len(i) for i in idxs])

    # pair big-with-small so both group capacities stay tight
    order = np.argsort(-cnts)
    pairs = [(int(order[i]), int(order[E - 1 - i])) for i in range(NPAIR)]
    C1 = -(-int(cnts[order[0]]) // 8) * 8
    C2 = -(-int(max(cnts[e2_] for _, e2_ in pairs)) // 8) * 8
    caps = (C1, C2)

    if caps not in _cache:
        _cache[caps] = _build_nc(caps)
    nc = _cache[caps]

    # ---- inputs (bf16, pre-transposed, pre-packed) ----
    bf = ml_dtypes.bfloat16
    chunks = _chunk_sizes(C1) + _chunk_sizes(C2)
    xpacks = []
    for eA, eB in pairs:
        xT = np.zeros((D, C1 + C2), dtype=bf)
        xT[:, :cnts[eA]] = flat[idxs[eA]].T.astype(bf)
        xT[:, C1:C1 + cnts[eB]] = flat[idxs[eB]].T.astype(bf)
        xpacks.append(_pack_x(xT, chunks))

    in_maps = []
    for c in range(E):
        p, hh = c // TP, c % TP
        r0 = hh * HS
        w13 = np.empty((2 * HS_T * 2, P, D_T * P), dtype=bf)
        w2s = np.empty((2, P, HS_T * D), dtype=bf)
        for slot, e in enumerate(pairs[p]):
            w1eT = np.ascontiguousarray(w1[e][r0:r0 + HS].T).astype(bf)  # [D, HS]
            w3eT = np.ascontiguousarray(w3[e][r0:r0 + HS].T).astype(bf)
            a1 = w1eT.reshape(D_T, P, HS).transpose(1, 0, 2)   # [128, D_T, HS]
            a3 = w3eT.reshape(D_T, P, HS).transpose(1, 0, 2)
            for s in range(HS_T):
                w13[slot * HS_T * 2 + s * 2 + 0] = \
                    a1[:, :, s * P:(s + 1) * P].reshape(P, D_T * P)
                w13[slot * HS_T * 2 + s * 2 + 1] = \
                    a3[:, :, s * P:(s + 1) * P].reshape(P, D_T * P)
            w2eT = np.ascontiguousarray(w2[e].T[r0:r0 + HS]).astype(bf)  # [HS, D]
            w2s[slot] = w2eT.reshape(HS_T, P, D).transpose(1, 0, 2).reshape(P, HS_T * D)
        in_maps.append({"xT": xpacks[p], "w13T": w13, "w2T": w2s})

    trace = os.environ.get("KERNEL_TRACE", "0") == "1"
    kwargs = {}
    if trace:
        kwargs = dict(trace=True, trace_cores=list(range(E)))
    res = run_bass_kernel_spmd(nc, in_maps, core_ids=list(range(E)), **kwargs)
    LAST_RESULTS = res

    # ---- combine: sum the two hidden-half partials, weight, scatter ----
    out = np.zeros((T, D), dtype=np.float32)
    for p, (eA, eB) in enumerate(pairs):
        ysum = (res.results[p * TP]["yT"].astype(np.float32)
                + res.results[p * TP + 1]["yT"].astype(np.float32))
        out[idxs[eA]] += wts[eA][:, None] * ysum[:, :cnts[eA]].T
        out[idxs[eB]] += wts[eB][:, None] * ysum[:, C1:C1 + cnts[eB]].T
    return out.reshape(B, S, D)
